# revision 11
# baseline (speedup 1.0000x reference)
"""3-level db4 wavelet low/high split for (32, 64, 16384) fp32 on 8 TRN2 NeuronCores.

Math: the reference computes wavedec (3-level db4, symmetric padding), then two
waverecs: `low` (details zeroed) and `high` (approximation zeroed).  Wavelets
give perfect reconstruction, so low + high == x and only the lowpass path is
needed: low = G @ (H @ x_row) with H (2054 x 16384) the composite 3-level
lowpass analysis operator (symmetric extension folded in) and G (16384 x 2054)
the lowpass synthesis operator; high = x - low on-chip.

Both operators are banded and 8-periodic, so all their 128-wide tiles dedupe
to 11 (stage 1) + 3 (stage 2) distinct weight tiles.

The kernel is HBM-bandwidth-bound, so all DRAM traffic is bf16 (tolerance is
2e-2; measured end-to-end bf16 error ~5e-3).  Matmul accumulation stays fp32
in PSUM.  DMA trigger instructions cost ~650 ns of sequencer time each, so
the two 128-row groups are packed side by side per partition in DRAM
([128, 2L] chunk-interleaved layout, host packs/unpacks): every transfer is a
single 2D DMA with 4 KB contiguous lines and one trigger moves 512 KB.

Device pipeline per core (256 rows = 2 row-groups of 128 partitions):
 - PE transpose-mode (bf16): x_row [rows, pos] -> x_sig [pos, rows] per block
 - stage 1 (PE, bf16): a3_sig[ab] = sum_pb HT_tile(pb,ab).T @ x_sig[pb]
 - stage 2 (PE, bf16): low_row = a3_sig-as-stationary @ GT_tile -> row-major
 - ACT: PSUM->SBUF bf16 cast of low;  DVE/Pool: high = x - low (all-bf16)
   and transpose-PSUM->SBUF copies, split to balance engine load
Sharding: batch*feature rows 2048 -> 256 rows per core, zero communication.
"""

import numpy as np
import ml_dtypes
import scipy.sparse as sp

import concourse.bacc as bacc
import concourse.tile as tile
from concourse import mybir
from concourse.bass_utils import run_bass_kernel_spmd

F32 = mybir.dt.float32
BF16 = mybir.dt.bfloat16
NP_BF16 = ml_dtypes.bfloat16

DEC_LO = np.array([-0.010597401785069032, 0.032883011666982945, 0.030841381835986965,
                   -0.18703481171888114, -0.02798376941698385, 0.6308807679295904,
                   0.7148465705525415, 0.23037781330885523], dtype=np.float64)
REC_LO = DEC_LO[::-1].copy()
F = 8
N_CORES = 8


def _symidx(n):
    idx = np.concatenate([np.arange(6, -1, -1), np.arange(n), np.arange(n - 1, n - 8, -1)])
    return idx[1:]


def _dwt_lo_mat(n):
    ext_idx = _symidx(n)
    lout = (n + 13 - F) // 2 + 1
    filt = DEC_LO[::-1]
    rows = np.repeat(np.arange(lout), F)
    cols = ext_idx[(2 * np.arange(lout)[:, None] + np.arange(F)[None, :]).ravel()]
    vals = np.tile(filt, lout)
    return sp.coo_matrix((vals, (rows, cols)), shape=(lout, n)).tocsr()


def _idwt_lo_mat(n):
    lout = 2 * n + 1 - F + 1
    filt = REC_LO[::-1]
    rows, cols, vals = [], [], []
    i = np.arange(lout)
    for k in range(F):
        pos = i + k
        m = (pos % 2 == 1)
        rows.append(i[m])
        cols.append((pos[m] - 1) // 2)
        vals.append(np.full(int(m.sum()), filt[k]))
    return sp.coo_matrix(
        (np.concatenate(vals), (np.concatenate(rows), np.concatenate(cols))),
        shape=(lout, n)).tocsr()


def _build_H_G(L, level=3):
    H = sp.identity(L, format="csr")
    lens = []
    n = L
    for _ in range(level):
        lens.append(n)
        D = _dwt_lo_mat(n)
        H = D @ H
        n = D.shape[0]
    G = sp.identity(n, format="csr")
    a_len = n
    for ln in lens[::-1]:
        d_len = (ln + F - 1) // 2
        if a_len == d_len + 1:
            G = sp.identity(a_len, format="csr")[:-1] @ G
            a_len -= 1
        U = _idwt_lo_mat(a_len)
        G = U @ G
        a_len = U.shape[0]
    return H, G


def _build_plan(L):
    H, G = _build_H_G(L)
    na = H.shape[0]
    nab = (na + 127) // 128
    HTp = np.zeros((L, nab * 128), np.float32)
    HTp[:, :na] = np.asarray(H.T.todense(), np.float32)
    GTp = np.zeros((nab * 128, L), np.float32)
    GTp[:na, :] = np.asarray(G.T.todense(), np.float32)

    npb = L // 128
    nw = L // 512

    s1_tiles, s1map = {}, [[] for _ in range(nab)]
    for ab in range(nab):
        for pb in range(npb):
            t = HTp[128 * pb:128 * pb + 128, 128 * ab:128 * ab + 128]
            if np.any(t):
                tid = s1_tiles.setdefault(t.tobytes(), len(s1_tiles))
                s1map[ab].append((pb, tid))
    w1 = np.zeros((128, 128 * len(s1_tiles)), np.float32)
    for key, tid in s1_tiles.items():
        w1[:, 128 * tid:128 * tid + 128] = np.frombuffer(key, np.float32).reshape(128, 128)

    s2_tiles, s2map = {}, [[] for _ in range(nw)]
    for ab in range(nab):
        for w in range(nw):
            t = GTp[128 * ab:128 * ab + 128, 512 * w:512 * w + 512]
            if np.any(t):
                nzc = np.nonzero(np.any(t != 0, axis=0))[0]
                tid = s2_tiles.setdefault(t.tobytes(), len(s2_tiles))
                s2map[w].append((ab, tid, int(nzc.min()), int(nzc.max()) + 1))
    for w in range(nw):
        s2map[w].sort(key=lambda e: -(e[3] - e[2]))  # widest (full-bank) first
    w2 = np.zeros((128, 512 * len(s2_tiles)), np.float32)
    for key, tid in s2_tiles.items():
        w2[:, 512 * tid:512 * tid + 512] = np.frombuffer(key, np.float32).reshape(128, 512)

    first_need = {}
    for ab in range(nab):
        for pb, _ in s1map[ab]:
            first_need.setdefault(pb, ab)
    tsched = [[] for _ in range(nab)]
    for pb, ab in first_need.items():
        tsched[ab].append(pb)
    # pairs of 512-wide output blocks -> one store of 1024 cols x both rgs
    npair = nw // 2
    psched = [[] for _ in range(nab)]
    for p in range(npair):
        kp = max(ab for w in (2 * p, 2 * p + 1) for ab, _, _, _ in s2map[w])
        psched[kp].append(p)
    for lst in tsched:
        lst.sort()
    for lst in psched:
        lst.sort()

    return dict(L=L, nab=nab, npb=npb, nw=nw, npair=npair, w1=w1, w2=w2,
                s1map=s1map, s2map=s2map, tsched=tsched, psched=psched)


def _build_program(plan, rows, xsig_bufs=16, xchunk_bufs=7, out_bufs=4):
    L, nab = plan["L"], plan["nab"]
    nrg = rows // 128
    assert rows % 128 == 0 and nrg == 2
    # DRAM layout [128, 2L]: logical chunk c (1024 cols, both rgs) lives at
    # packed cols [2048c, 2048c+2048) = rg0 1024 | rg1 1024.  16 chunks.
    nch = L // 1024

    nc = bacc.Bacc("TRN2", target_bir_lowering=False, debug=False)
    x_d = nc.dram_tensor("x", [128, 2 * L], BF16, kind="ExternalInput").ap()
    w1_d = nc.dram_tensor("w1", list(plan["w1"].shape), BF16, kind="ExternalInput").ap()
    w2_d = nc.dram_tensor("w2", list(plan["w2"].shape), BF16, kind="ExternalInput").ap()
    id_d = nc.dram_tensor("ident", [128, 128], BF16, kind="ExternalInput").ap()
    low_d = nc.dram_tensor("low", [128, 2 * L], BF16, kind="ExternalOutput").ap()
    high_d = nc.dram_tensor("high", [128, 2 * L], BF16, kind="ExternalOutput").ap()

    with tile.TileContext(nc) as tc:
        with tc.tile_pool(name="sbw", bufs=1) as sbw, \
             tc.tile_pool(name="sbx", bufs=xchunk_bufs) as sbx, \
             tc.tile_pool(name="sbxs", bufs=xsig_bufs) as sbxs, \
             tc.tile_pool(name="sba3", bufs=nab) as sba3, \
             tc.tile_pool(name="sbo", bufs=out_bufs) as sbo, \
             tc.tile_pool(name="pst", bufs=3, space="PSUM") as pst, \
             tc.tile_pool(name="psa", bufs=1, space="PSUM") as psa, \
             tc.tile_pool(name="ps2", bufs=2, space="PSUM") as ps2p:

            idt = sbw.tile([128, 128], BF16, tag="idt")
            nc.sync.dma_start(idt[:], id_d[:])
            w1t = sbw.tile(list(plan["w1"].shape), BF16, tag="w1t")
            wq = plan["w1"].shape[1] // 2
            for i in range(2):
                nc.sync.dma_start(w1t[:, i * wq:(i + 1) * wq], w1_d[:, i * wq:(i + 1) * wq])
            w2t = sbw.tile(list(plan["w2"].shape), BF16, tag="w2t")
            wq2 = plan["w2"].shape[1] // 2
            for i in range(2):
                nc.sync.dma_start(w2t[:, i * wq2:(i + 1) * wq2], w2_d[:, i * wq2:(i + 1) * wq2])

            xch, xsig, a3 = {}, {}, {}
            chunks_issued = set()
            ncopy = 0
            nsub = 0

            def ensure_chunk(c):
                if c in chunks_issued or c >= nch:
                    return
                chunks_issued.add(c)
                xt = sbx.tile([128, 2048], BF16, tag="x")
                # quarter the first chunks across queues: first-data latency
                # beats trigger cost at pipeline start
                nsplit = 4 if c < 3 else 1
                step = 2048 // nsplit
                for i in range(nsplit):
                    nc.gpsimd.dma_start(
                        xt[:, i * step:(i + 1) * step],
                        x_d[:, 2048 * c + i * step:2048 * c + (i + 1) * step])
                xch[c] = xt

            for k in range(nab):
                for pb in plan["tsched"][k]:
                    c = pb // 8
                    ensure_chunk(c)
                    ensure_chunk(c + 1)
                    off = (pb % 8) * 128
                    pt = pst.tile([128, 128 * nrg], BF16, tag="pt")
                    for rg in range(nrg):
                        nc.tensor.transpose(
                            pt[:, rg * 128:(rg + 1) * 128],
                            xch[c][:, rg * 1024 + off:rg * 1024 + off + 128], idt[:])
                    xs = sbxs.tile([128, 128 * nrg], BF16, tag="xs")
                    nc.vector.tensor_copy(xs[:], pt[:])
                    ncopy += 1
                    xsig[pb] = xs

                pa = psa.tile([128, 128 * nrg], F32, tag="pa")
                ents = plan["s1map"][k]
                for i, (pb, tid) in enumerate(ents):
                    nc.tensor.matmul(
                        pa[:], w1t[:, 128 * tid:128 * tid + 128], xsig[pb][:],
                        start=(i == 0), stop=(i == len(ents) - 1))
                a3t = sba3.tile([128, 128 * nrg], BF16, tag="a3")
                nc.scalar.copy(a3t[:], pa[:])
                a3[k] = a3t

                for p in plan["psched"][k]:
                    c = p  # pair p covers logical cols [1024p, 1024p+1024) == chunk p
                    lo = sbo.tile([128, 2048], BF16, tag="lo")
                    hi = sbo.tile([128, 2048], BF16, tag="hi")
                    for rg in range(nrg):
                        for h in range(2):
                            w = 2 * p + h
                            po = ps2p.tile([128, 512], F32, tag=f"s2r{rg}")
                            ents2 = plan["s2map"][w]
                            for j, (ab, tid, clo, chi) in enumerate(ents2):
                                nc.tensor.matmul(
                                    po[:, clo:chi],
                                    a3[ab][:, rg * 128:(rg + 1) * 128],
                                    w2t[:, 512 * tid + clo:512 * tid + chi],
                                    start=(j == 0), stop=(j == len(ents2) - 1))
                            sl = slice(rg * 1024 + 512 * h, rg * 1024 + 512 * h + 512)
                            nc.scalar.copy(lo[:, sl], po[:])
                            # high = x - low, all-bf16 SBUF operands; Pool takes a
                            # 1-in-4 share (it runs ~2x below roofline on Q7)
                            if nsub % 4 == 3:
                                nc.gpsimd.tensor_sub(hi[:, sl], xch[c][:, sl], lo[:, sl])
                            else:
                                nc.vector.tensor_sub(hi[:, sl], xch[c][:, sl], lo[:, sl])
                            nsub += 1
                    # halve the final pairs' stores: shrinks the drain tail
                    nsplit = 2 if p >= plan["npair"] - 2 else 1
                    step = 2048 // nsplit
                    for i in range(nsplit):
                        nc.sync.dma_start(
                            low_d[:, 2048 * p + i * step:2048 * p + (i + 1) * step],
                            lo[:, i * step:(i + 1) * step])
                        nc.sync.dma_start(
                            high_d[:, 2048 * p + i * step:2048 * p + (i + 1) * step],
                            hi[:, i * step:(i + 1) * step])

    nc.compile()
    return nc


_CACHE = {}


def _get_program(rows, L):
    key = (rows, L)
    if key not in _CACHE:
        plan = _build_plan(L)
        nc = _build_program(plan, rows=rows)
        _CACHE[key] = (plan, nc)
    return _CACHE[key]


def _pack(xc, L):
    # [256, L] -> [128, 2L] chunk-interleaved: row p = [rg0 c0 | rg1 c0 | rg0 c1 | ...]
    return np.ascontiguousarray(
        xc.reshape(2, 128, L // 1024, 1024).transpose(1, 2, 0, 3).reshape(128, 2 * L))


def _unpack(yc, L):
    # inverse of _pack: [128, 2L] -> [256, L]
    return yc.reshape(128, L // 1024, 2, 1024).transpose(2, 0, 1, 3).reshape(256, L)


def _make_in_maps(x):
    x = np.asarray(x)
    B, Fd, L = x.shape
    xf = np.ascontiguousarray(x.reshape(B * Fd, L)).astype(NP_BF16)
    rows = (B * Fd) // N_CORES
    plan, nc = _get_program(rows, L)
    w1 = plan["w1"].astype(NP_BF16)
    w2 = plan["w2"].astype(NP_BF16)
    ident = np.eye(128, dtype=np.float32).astype(NP_BF16)
    in_maps = [{
        "x": _pack(xf[c * rows:(c + 1) * rows], L),
        "w1": w1, "w2": w2, "ident": ident,
    } for c in range(N_CORES)]
    return in_maps, nc, (B, Fd, L)


def kernel(x):
    in_dtype = np.asarray(x).dtype
    in_maps, nc, (B, Fd, L) = _make_in_maps(x)
    res = run_bass_kernel_spmd(nc, in_maps, list(range(N_CORES)))
    low = np.concatenate([_unpack(np.asarray(r["low"]), L) for r in res.results], axis=0)
    high = np.concatenate([_unpack(np.asarray(r["high"]), L) for r in res.results], axis=0)
    low = low.astype(np.float32).reshape(B, Fd, L)
    high = high.astype(np.float32).reshape(B, Fd, L)
    return low.astype(in_dtype, copy=False), high.astype(in_dtype, copy=False)
